# revision 4
# baseline (speedup 1.0000x reference)
"""Cross-attention reducer kernel for Trainium2, 8 NeuronCores (SPMD).

Problem (full shapes):
    token_input    [T=8192, L=4096]
    learned_queries[V=4096, I=512]
    w_q [I, I], w_k [L, I], w_v [L, I], w_out [I, L]

    q = learned_queries @ w_q;  k = token_input @ w_k;  v = token_input @ w_v
    per head h (H=8, D=64): attn = softmax(q_h k_h^T / sqrt(D)); out_h = attn @ v_h
    out = concat_h(out_h) @ w_out      -> [V, L]

Sharding: queries (V) are sharded 8 ways; the K/V projections are
sequence-parallel (each core projects its T/8 token shard) followed by an
AllGather of k^T and v, after which every core runs attention for all 8 heads
over its own 512 queries and the full gathered T, then applies the output
projection for its V-shard. Everything is computed transposed
(final^T = w_out^T-contraction) so every matmul contracts on the partition
dimension with no large transposes anywhere:

    q^T  [I, Vs]  = w_q (lhsT)  x lq^T (rhs)
    k^T  [I, t]   = w_k (lhsT)  x tok^T (rhs)        (gathered)
    v^T  [I, t]   = w_v (lhsT)  x tok^T (rhs), then 128x128 PE-transposes
                    to v [t, I] before the gather
    s^T  [t, Vs]  = k_h^T (lhsT) x q_h^T (rhs)       (t-tiles of 128)
    p^T           = exp(s^T / 8)                      (no max-subtraction:
                    scores are O(3), exp can't overflow; identical math)
    u^T  [D+1,Vs] = [v_h | 1] (lhsT) x p^T (rhs)     (row D = softmax denom)
    a^T  [D, Vs]  = u^T * (1/denom broadcast via PE outer product)
    out^T[L, Vs]  = w_out (lhsT) x a^T (rhs)

All inputs are cast to bf16 on the host (input rounding contributes ~3e-3
max-rel error, tolerance 2e-2); all matmuls run bf16 at 1 cycle/col.

Overlap structure (the point of this version):
  - q projection runs first, under the token-shard load.
  - token shard loaded ONCE into SBUF (bf16), reused by K and V projections.
  - each gather is split into two t-half collectives kicked at the
    projection midpoints, so collective transfer is pipelined with the
    remaining projection work; gathered v is loaded in quarters and the
    attention processes half-a tiles before half-b tiles, so the first
    attn@v only needs the first quarter of the first v-gather.
  - the per-head softmax normalization (denominator broadcast) is deferred
    into the NEXT head's score phase so its serial chain (shift ->
    reciprocal -> PE broadcast) never blocks the in-order tensor queue.
  - collectives + gather-dependent loads (vh_all, w_out prefetch, aT
    stores) issue on gpsimd; all other loads on sync, so a blocking
    collective can never stall the compute-feed queues.
"""

import os

import numpy as np
import ml_dtypes

import concourse.bacc as bacc
import concourse.tile as tile
import concourse.mybir as mybir
from concourse.bass_utils import run_bass_kernel_spmd

F32 = mybir.dt.float32
BF16 = mybir.dt.bfloat16
EXP = mybir.ActivationFunctionType.Exp
EQ = mybir.AluOpType.is_equal

N_CORES = 8
T, L, V, INNER = 8192, 4096, 4096, 512
H, D = 8, 64
TS = T // N_CORES      # 1024  t-shard per core
QS = V // N_CORES      # 512   query shard per core
SCALE = D ** -0.5      # 0.125

NT = T // 128          # 64 gathered t-tiles per head
GRP = 3                # t-tiles per exp batch (3 psum banks)

# processing position o (0..63) -> (th half, core, in-half tile 0..3)
# matches the [half, core, tile] layout of the gathered buffers
ORDER = [(o // 32, (o % 32) // 4, o % 4) for o in range(64)]


def build_program():
    nc = bacc.Bacc(
        "TRN2", target_bir_lowering=False, debug=False, num_devices=N_CORES
    )

    tok_T = nc.dram_tensor("tok_T", [L, TS], BF16, kind="ExternalInput").ap()
    lq_T = nc.dram_tensor("lq_T", [INNER, QS], BF16, kind="ExternalInput").ap()
    w_q = nc.dram_tensor("w_q", [INNER, INNER], BF16, kind="ExternalInput").ap()
    w_k = nc.dram_tensor("w_k", [L, INNER], BF16, kind="ExternalInput").ap()
    w_v = nc.dram_tensor("w_v", [L, INNER], BF16, kind="ExternalInput").ap()
    w_out = nc.dram_tensor("w_out", [INNER, L], BF16, kind="ExternalInput").ap()
    outT = nc.dram_tensor("outT", [L, QS], F32, kind="ExternalOutput").ap()

    # rearranged DRAM views (partition-major for SBUF loads)
    tok_v = tok_T.rearrange("(k p) t -> p k t", p=128)          # [128, 32, 1024]
    lq_v = lq_T.rearrange("(k p) q -> p k q", p=128)            # [128, 4, 512]
    w_q_v = w_q.rearrange("(k p) i -> p k i", p=128)            # [128, 4, 512]
    w_k_v = w_k.rearrange("(k p) i -> p k i", p=128)            # [128, 32, 512]
    w_v_v = w_v.rearrange("(k p) i -> p k i", p=128)            # [128, 32, 512]
    w_out_v = w_out.rearrange("(k p) l -> p k l", p=128)        # [128, 4, 4096]

    no_cc = bool(os.environ.get("BASSK_NO_CC"))

    with tile.TileContext(nc) as tc:
        with (
            tc.tile_pool(name="persist", bufs=1) as persist,
            tc.tile_pool(name="dram", bufs=1, space="DRAM") as dram,
        ):
            # ---- persistent SBUF across phases ----
            qT_sb = persist.tile([64, H, QS], BF16, tag="qT")        # q^T per head
            aT_sb = persist.tile([128, 4, QS], BF16, tag="aT")       # attn out^T
            idn = persist.tile([128, 128], BF16, tag="idn")          # identity
            ones_64 = persist.tile([1, D], F32, tag="ones64")

            # collective bounce buffers, one pair per t-half
            gk_in = [dram.tile([INNER, 512], BF16, tag=f"gk_in{t}") for t in range(2)]
            gk_out = [
                dram.tile([N_CORES * INNER, 512], BF16, tag=f"gk_out{t}",
                          addr_space="Shared")
                for t in range(2)
            ]
            gv_in = [dram.tile([512, INNER], BF16, tag=f"gv_in{t}") for t in range(2)]
            gv_out = [
                dram.tile([N_CORES * 512, INNER], BF16, tag=f"gv_out{t}",
                          addr_space="Shared")
                for t in range(2)
            ]
            gk_in_v = [g.rearrange("(m p) t -> p m t", p=128) for g in gk_in]
            gv_in_v = [g.rearrange("(j p) i -> p j i", p=128) for g in gv_in]

            def gather(src, dst):
                if no_cc:
                    nc.sync.dma_start(dst[0:src.shape[0], :], src[:])
                else:
                    nc.gpsimd.collective_compute(
                        "AllGather", mybir.AluOpType.bypass,
                        replica_groups=[list(range(N_CORES))],
                        ins=[src.opt()], outs=[dst.opt()],
                    )

            # identity matrix for PE transposes: idn[p, f] = (f == p)
            with tc.tile_pool(name="idpool", bufs=1) as idp:
                irow = idp.tile([128, 128], F32, tag="irow")
                icol = idp.tile([128, 1], F32, tag="icol")
                nc.gpsimd.iota(irow[:], pattern=[[1, 128]], base=0, channel_multiplier=0, allow_small_or_imprecise_dtypes=True)
                nc.gpsimd.iota(icol[:], pattern=[[0, 1]], base=0, channel_multiplier=1, allow_small_or_imprecise_dtypes=True)
                nc.vector.tensor_scalar(idn[:], irow[:], icol[:], None, EQ)
            nc.vector.memset(ones_64[:], 1.0)

            # ================= phase 1: projections =================
            with (
                tc.tile_pool(name="ptok", bufs=1) as ptok,
                tc.tile_pool(name="proj", bufs=2) as proj,
                tc.tile_pool(name="pps", bufs=2, space="PSUM") as pps,
            ):
                # --- q^T projection first (PE warms up under the tok load) ---
                wq_sb = proj.tile([128, 4, INNER], BF16, tag="wq", bufs=1)
                lq_sb = proj.tile([128, 4, QS], BF16, tag="lq", bufs=1)
                nc.gpsimd.dma_start(wq_sb[:], w_q_v)
                nc.gpsimd.dma_start(lq_sb[:], lq_v)

                # token shard resident in SBUF, loaded once (2 chunked DMAs)
                tok_sb = ptok.tile([128, 32, TS], BF16, tag="tok")
                nc.sync.dma_start(tok_sb[:, :, 0:512], tok_v[:, :, 0:512])
                nc.sync.dma_start(tok_sb[:, :, 512:1024], tok_v[:, :, 512:1024])

                for m in range(4):
                    ps = pps.tile([128, QS], F32, tag="pp")
                    for kk in range(4):
                        nc.tensor.matmul(
                            ps[:],
                            wq_sb[:, kk, m * 128:(m + 1) * 128],
                            lq_sb[:, kk, :],
                            start=(kk == 0), stop=(kk == 3),
                        )
                    qstage = proj.tile([128, QS], BF16, tag="qstage")
                    nc.vector.tensor_copy(qstage[:], ps[:])
                    # shift each head's 64 rows down to base partition 0
                    nc.gpsimd.dma_start(qT_sb[:, 2 * m, :], qstage[0:64, :])
                    nc.gpsimd.dma_start(qT_sb[:, 2 * m + 1, :], qstage[64:128, :])

                # --- k^T projection per t-half; gather kicked per half ---
                for th in range(2):
                    for m in range(4):
                        wcol = proj.tile([128, 32, 128], BF16, tag="wcol")
                        nc.sync.dma_start(wcol[:], w_k_v[:, :, m * 128:(m + 1) * 128])
                        ps = pps.tile([128, 512], F32, tag="pp")
                        for k in range(32):
                            nc.tensor.matmul(
                                ps[:], wcol[:, k, :], tok_sb[:, k, th * 512:(th + 1) * 512],
                                start=(k == 0), stop=(k == 31),
                            )
                        kstage = proj.tile([128, 512], BF16, tag="kstage")
                        nc.vector.tensor_copy(kstage[:], ps[:])
                        nc.sync.dma_start(gk_in_v[th][:, m, :], kstage[:])
                    gather(gk_in[th], gk_out[th])

                # --- v^T projection + PE transpose per t-half; gathered ---
                for th in range(2):
                    for m in range(4):
                        wcol = proj.tile([128, 32, 128], BF16, tag="wcol")
                        nc.sync.dma_start(wcol[:], w_v_v[:, :, m * 128:(m + 1) * 128])
                        ps = pps.tile([128, 512], F32, tag="pp")
                        for k in range(32):
                            nc.tensor.matmul(
                                ps[:], wcol[:, k, :], tok_sb[:, k, th * 512:(th + 1) * 512],
                                start=(k == 0), stop=(k == 31),
                            )
                        vst = proj.tile([128, 512], BF16, tag="vst")
                        nc.vector.tensor_copy(vst[:], ps[:])
                        pt = pps.tile([128, 512], BF16, tag="pt")
                        for j in range(4):
                            nc.tensor.transpose(
                                pt[:, j * 128:(j + 1) * 128],
                                vst[:, j * 128:(j + 1) * 128],
                                idn[:],
                            )
                        # pt columns j hold v[t-chunk j of this half, i-block m]
                        vstage = proj.tile([128, 4, 128], BF16, tag="vstage")
                        nc.vector.tensor_copy(
                            vstage[:], pt[:].rearrange("p (j i) -> p j i", j=4)
                        )
                        nc.sync.dma_start(
                            gv_in_v[th][:, :, m * 128:(m + 1) * 128], vstage[:]
                        )
                    gather(gv_in[th], gv_out[th])

            # ================= phase 2: attention =================
            # gathered views: k per head row-slice; v in processing order
            gk_head = [
                g.rearrange("(c p) t -> p c t", p=INNER) for g in gk_out
            ]                                                       # [512, 8, 512]
            gv_v = [
                g.rearrange("(x p) i -> p x i", p=128) for g in gv_out
            ]                                                       # [128, 32, 512]
            groups = [list(range(s, min(s + GRP, NT))) for s in range(0, NT, GRP)]

            with (
                tc.tile_pool(name="attn", bufs=2) as attn,
                tc.tile_pool(name="attn3", bufs=8) as attn3,
                tc.tile_pool(name="aps", bufs=2, space="PSUM") as aps,
                tc.tile_pool(name="aps1", bufs=1, space="PSUM") as aps1,
            ):
                # all heads' V, position-ordered, loaded in quarters (gpsimd:
                # queued behind the matching gather; quarter q covers
                # positions 16q..16q+16 = cores 4q'..4q'+4 of half q//2)
                vh_all = attn.tile([128, NT, INNER], BF16, tag="vh_all", bufs=1)
                for q in range(4):
                    nc.gpsimd.dma_start(
                        vh_all[:, q * 16:(q + 1) * 16, :],
                        gv_v[q // 2][:, (q % 2) * 16:(q % 2) * 16 + 16, :],
                    )
                # w_out prefetch (no gather dep, but gpsimd is free now)
                wo_all = attn.tile([128, 4, L], BF16, tag="wo_all", bufs=1)
                nc.gpsimd.dma_start(wo_all[:], w_out_v)

                def prefetch_head(h):
                    kTh = attn.tile([64, 2, N_CORES, 512], BF16, tag="kTh")
                    for t in range(2):
                        nc.sync.dma_start(
                            kTh[:, t, :, :], gk_head[t][h * D:(h + 1) * D, :, :]
                        )
                    vh = attn.tile([128, NT, D + 1], BF16, tag="vh")
                    nc.vector.memset(vh[:, :, D], 1.0)
                    for q in range(4):
                        nc.vector.tensor_copy(
                            vh[:, q * 16:(q + 1) * 16, 0:D],
                            vh_all[:, q * 16:(q + 1) * 16, h * D:(h + 1) * D],
                        )
                    return kTh, vh

                def norm_tail(h, u_sb, recip):
                    """PE broadcast of 1/denom + rescale + store (deferrable)."""
                    ps_r = aps.tile([D, QS], F32, tag="ps_s")  # borrow a slot
                    nc.tensor.matmul(ps_r[:], ones_64[:], recip[:], start=True, stop=True)
                    a_tmp = attn.tile([D, QS], BF16, tag="a_tmp")
                    nc.vector.tensor_mul(a_tmp[:], u_sb[0:D, :], ps_r[:])
                    nc.gpsimd.dma_start(
                        aT_sb[(h % 2) * 64:(h % 2) * 64 + 64, h // 2, :], a_tmp[:]
                    )

                nxt = prefetch_head(0)
                deferred = None  # (h, u_sb, recip) of previous head
                for h in range(H):
                    kTh, vh = nxt
                    qTh = qT_sb[:, h, :]
                    ps_o = aps1.tile([D + 1, QS], F32, tag="ps_o")
                    lag = 7 if h == 0 else 2
                    pending = []
                    for gi, g in enumerate(groups):
                        ps_s = aps.tile([128, GRP * QS], F32, tag="ps_s")
                        for jj, o in enumerate(g):
                            t, c, jt = ORDER[o]
                            nc.tensor.matmul(
                                ps_s[:, jj * QS:(jj + 1) * QS],
                                kTh[:, t, c, jt * 128:(jt + 1) * 128],
                                qTh,
                                start=True, stop=True,
                            )
                        pT = attn3.tile([128, GRP * QS], BF16, tag="pT")
                        n = len(g) * QS
                        nc.scalar.activation(pT[:, 0:n], ps_s[:, 0:n], EXP, scale=SCALE)
                        pending.append((g, pT))
                        if gi == 2 and h + 1 < H:
                            nxt = prefetch_head(h + 1)
                        if gi == 3 and deferred is not None:
                            norm_tail(*deferred)
                            deferred = None
                        if len(pending) > lag:
                            pg, ppT = pending.pop(0)
                            for jj, o in enumerate(pg):
                                nc.tensor.matmul(
                                    ps_o[:], vh[:, o, :], ppT[:, jj * QS:(jj + 1) * QS],
                                    start=(o == 0), stop=(o == NT - 1),
                                    skip_group_check=True,
                                )
                    for pg, ppT in pending:
                        for jj, o in enumerate(pg):
                            nc.tensor.matmul(
                                ps_o[:], vh[:, o, :], ppT[:, jj * QS:(jj + 1) * QS],
                                start=(o == 0), stop=(o == NT - 1),
                                skip_group_check=True,
                            )

                    # u^T and 1/denom now; the broadcast+rescale is deferred
                    # into the next head's score phase
                    u_sb = attn.tile([D + 1, QS], F32, tag="u")
                    nc.vector.tensor_copy(u_sb[:], ps_o[:])
                    dn0 = attn.tile([1, QS], F32, tag="dn0")
                    nc.sync.dma_start(dn0[:], u_sb[D:D + 1, :])  # to partition 0
                    recip = attn.tile([1, QS], F32, tag="recip")
                    nc.vector.reciprocal(recip[:], dn0[:])
                    deferred = (h, u_sb, recip)
                norm_tail(*deferred)

                # ============ phase 3: output projection ============
                for m in range(L // 128):
                    ps = aps.tile([128, QS], F32, tag="ps_s")
                    for kk in range(4):
                        nc.tensor.matmul(
                            ps[:], wo_all[:, kk, m * 128:(m + 1) * 128],
                            aT_sb[:, kk, :],
                            start=(kk == 0), stop=(kk == 3),
                        )
                    of = attn.tile([128, QS], F32, tag="of", bufs=3)
                    nc.vector.tensor_copy(of[:], ps[:])
                    nc.sync.dma_start(outT[m * 128:(m + 1) * 128, :], of[:])

    nc.compile()
    return nc


_COMPILED = None


def _get_compiled():
    global _COMPILED
    if _COMPILED is None:
        _COMPILED = build_program()
    return _COMPILED


def _bf(x):
    return np.ascontiguousarray(np.asarray(x, dtype=np.float32)).astype(
        ml_dtypes.bfloat16
    )


def make_in_maps(token_input, learned_queries, w_q, w_k, w_v, w_out):
    token_input = np.asarray(token_input, dtype=np.float32)
    learned_queries = np.asarray(learned_queries, dtype=np.float32)
    w_q_b, w_k_b, w_v_b, w_out_b = _bf(w_q), _bf(w_k), _bf(w_v), _bf(w_out)
    in_maps = []
    for c in range(N_CORES):
        in_maps.append({
            "tok_T": _bf(token_input[c * TS:(c + 1) * TS, :].T),
            "lq_T": _bf(learned_queries[c * QS:(c + 1) * QS, :].T),
            "w_q": w_q_b, "w_k": w_k_b, "w_v": w_v_b, "w_out": w_out_b,
        })
    return in_maps


def assemble(results):
    out = np.empty((V, L), dtype=np.float32)
    for c in range(N_CORES):
        out[c * QS:(c + 1) * QS, :] = results[c]["outT"].T
    return out


def kernel(token_input, learned_queries, w_q, w_k, w_v, w_out):
    nc = _get_compiled()
    in_maps = make_in_maps(token_input, learned_queries, w_q, w_k, w_v, w_out)
    res = run_bass_kernel_spmd(nc, in_maps, list(range(N_CORES)))
    return assemble(res.results)


# revision 9
# speedup vs baseline: 5.3971x; 5.3971x over previous
"""Cross-attention reducer kernel for Trainium2, 8 NeuronCores (SPMD).

Problem (full shapes):
    token_input    [T=8192, L=4096]
    learned_queries[V=4096, I=512]
    w_q [I, I], w_k [L, I], w_v [L, I], w_out [I, L]

    q = learned_queries @ w_q;  k = token_input @ w_k;  v = token_input @ w_v
    per head h (H=8, D=64): attn = softmax(q_h k_h^T / sqrt(D)); out_h = attn @ v_h
    out = concat_h(out_h) @ w_out      -> [V, L]

Sharding: queries (V) are sharded 8 ways; the K/V projections are
sequence-parallel (each core projects its T/8 token shard) followed by an
AllGather of k^T and v, after which every core runs attention for all 8 heads
over its own 512 queries and the full gathered T, then applies the output
projection for its V-shard. Everything is computed transposed
(final^T = w_out^T-contraction) so every matmul contracts on the partition
dimension with no large transposes anywhere:

    q^T  [I, Vs]  = w_q (lhsT)  x lq^T (rhs)
    k^T  [I, t]   = w_k (lhsT)  x tok^T (rhs)        (gathered)
    v^T  [I, t]   = w_v (lhsT)  x tok^T (rhs), then 128x128 PE-transposes
                    to v [t, I] before the gather
    s^T  [t, Vs]  = k_h^T (lhsT) x q_h^T (rhs)       (t-tiles of 128)
    p^T           = exp(s^T / 8)                      (no max-subtraction:
                    scores are O(3), exp can't overflow; identical math)
    u^T  [D+1,Vs] = [v_h | 1] (lhsT) x p^T (rhs)     (row D = softmax denom)
    a^T  [D, Vs]  = u^T * (1/denom broadcast via PE outer product)
    out^T[L, Vs]  = w_out (lhsT) x a^T (rhs)

All inputs are cast to bf16 on the host (input rounding contributes ~3e-3
max-rel error, tolerance 2e-2); all matmuls run bf16 at 1 cycle/col.

Overlap structure (the point of this version):
  - q projection runs first, under the token-shard load.
  - token shard loaded ONCE into SBUF (bf16), reused by K and V projections.
  - each gather is split into two t-half collectives kicked at the
    projection midpoints, so collective transfer is pipelined with the
    remaining projection work; gathered v is loaded in quarters and the
    attention processes half-a tiles before half-b tiles, so the first
    attn@v only needs the first quarter of the first v-gather.
  - the per-head softmax normalization (denominator broadcast) is deferred
    into the NEXT head's score phase so its serial chain (shift ->
    reciprocal -> PE broadcast) never blocks the in-order tensor queue.
  - collectives + gather-dependent loads (vh_all, w_out prefetch, aT
    stores) issue on gpsimd; all other loads on sync, so a blocking
    collective can never stall the compute-feed queues.
"""

import os

import numpy as np
import ml_dtypes

import concourse.bacc as bacc
import concourse.tile as tile
import concourse.mybir as mybir
from concourse.bass_utils import run_bass_kernel_spmd

F32 = mybir.dt.float32
BF16 = mybir.dt.bfloat16
EXP = mybir.ActivationFunctionType.Exp
EQ = mybir.AluOpType.is_equal

N_CORES = 8
T, L, V, INNER = 8192, 4096, 4096, 512
H, D = 8, 64
TS = T // N_CORES      # 1024  t-shard per core
QS = V // N_CORES      # 512   query shard per core
SCALE = D ** -0.5      # 0.125

NT = T // 128          # 64 gathered t-tiles per head
GRP = 3                # t-tiles per exp batch (3 psum banks)

# processing position o (0..63) -> (th half, core, in-half tile 0..3)
# matches the [half, core, tile] layout of the gathered buffers
ORDER = [(o // 32, (o % 32) // 4, o % 4) for o in range(64)]


def build_program():
    nc = bacc.Bacc(
        "TRN2", target_bir_lowering=False, debug=False, num_devices=N_CORES
    )

    tok_T = nc.dram_tensor("tok_T", [L, TS], BF16, kind="ExternalInput").ap()
    lq_T = nc.dram_tensor("lq_T", [INNER, QS], BF16, kind="ExternalInput").ap()
    w_q = nc.dram_tensor("w_q", [INNER, INNER], BF16, kind="ExternalInput").ap()
    w_k = nc.dram_tensor("w_k", [L, INNER], BF16, kind="ExternalInput").ap()
    w_v = nc.dram_tensor("w_v", [L, INNER], BF16, kind="ExternalInput").ap()
    w_out = nc.dram_tensor("w_out", [INNER, L], BF16, kind="ExternalInput").ap()
    outT = nc.dram_tensor("outT", [L, QS], F32, kind="ExternalOutput").ap()

    # rearranged DRAM views (partition-major for SBUF loads)
    tok_v = tok_T.rearrange("(k p) t -> p k t", p=128)          # [128, 32, 1024]
    lq_v = lq_T.rearrange("(k p) q -> p k q", p=128)            # [128, 4, 512]
    w_q_v = w_q.rearrange("(k p) i -> p k i", p=128)            # [128, 4, 512]
    w_k_v = w_k.rearrange("(k p) i -> p k i", p=128)            # [128, 32, 512]
    w_v_v = w_v.rearrange("(k p) i -> p k i", p=128)            # [128, 32, 512]
    w_out_v = w_out.rearrange("(k p) l -> p k l", p=128)        # [128, 4, 4096]

    no_cc = bool(os.environ.get("BASSK_NO_CC"))

    with tile.TileContext(nc) as tc:
        with (
            tc.tile_pool(name="persist", bufs=1) as persist,
            tc.tile_pool(name="dram", bufs=1, space="DRAM") as dram,
        ):
            # ---- persistent SBUF across phases ----
            qT_sb = persist.tile([64, H, QS], BF16, tag="qT")        # q^T per head
            aT_sb = persist.tile([128, 4, QS], BF16, tag="aT")       # attn out^T
            idn = persist.tile([128, 128], BF16, tag="idn")          # identity
            ones_64 = persist.tile([1, D], F32, tag="ones64")

            # collective bounce buffers, one pair per t-half
            gk_in = [dram.tile([INNER, 512], BF16, tag=f"gk_in{t}", name=f"gk_in{t}") for t in range(2)]
            gk_out = [
                dram.tile([N_CORES * INNER, 512], BF16, tag=f"gk_out{t}",
                          name=f"gk_out{t}", addr_space="Shared")
                for t in range(2)
            ]
            gv_in = [dram.tile([512, INNER], BF16, tag=f"gv_in{t}", name=f"gv_in{t}") for t in range(2)]
            gv_out = [
                dram.tile([N_CORES * 512, INNER], BF16, tag=f"gv_out{t}",
                          name=f"gv_out{t}", addr_space="Shared")
                for t in range(2)
            ]
            gk_in_v = [g.rearrange("(m p) t -> p m t", p=128) for g in gk_in]
            gv_in_v = [g.rearrange("(j p) i -> p j i", p=128) for g in gv_in]

            def gather(src, dst):
                if no_cc:
                    nc.sync.dma_start(dst[0:src.shape[0], :], src[:])
                else:
                    nc.gpsimd.collective_compute(
                        "AllGather", mybir.AluOpType.bypass,
                        replica_groups=[list(range(N_CORES))],
                        ins=[src.opt()], outs=[dst.opt()],
                    )

            # identity matrix for PE transposes: idn[p, f] = (f == p)
            with tc.tile_pool(name="idpool", bufs=1) as idp:
                irow = idp.tile([128, 128], F32, tag="irow")
                icol = idp.tile([128, 1], F32, tag="icol")
                nc.gpsimd.iota(irow[:], pattern=[[1, 128]], base=0, channel_multiplier=0, allow_small_or_imprecise_dtypes=True)
                nc.gpsimd.iota(icol[:], pattern=[[0, 1]], base=0, channel_multiplier=1, allow_small_or_imprecise_dtypes=True)
                nc.vector.tensor_scalar(idn[:], irow[:], icol[:], None, EQ)
            nc.vector.memset(ones_64[:], 1.0)

            # ================= phase 1: projections =================
            with (
                tc.tile_pool(name="ptok", bufs=1) as ptok,
                tc.tile_pool(name="proj", bufs=2) as proj,
                tc.tile_pool(name="pps", bufs=2, space="PSUM") as pps,
            ):
                # --- q^T projection first (PE warms up under the tok load) ---
                wq_sb = proj.tile([128, 4, INNER], BF16, tag="wq", bufs=1)
                lq_sb = proj.tile([128, 4, QS], BF16, tag="lq", bufs=1)
                nc.gpsimd.dma_start(wq_sb[:], w_q_v)
                nc.gpsimd.dma_start(lq_sb[:], lq_v)

                # token shard resident in SBUF, loaded once (2 chunked DMAs)
                tok_sb = ptok.tile([128, 32, TS], BF16, tag="tok")
                nc.sync.dma_start(tok_sb[:, :, 0:512], tok_v[:, :, 0:512])
                nc.sync.dma_start(tok_sb[:, :, 512:1024], tok_v[:, :, 512:1024])

                for m in range(4):
                    ps = pps.tile([128, QS], F32, tag="pp")
                    for kk in range(4):
                        nc.tensor.matmul(
                            ps[:],
                            wq_sb[:, kk, m * 128:(m + 1) * 128],
                            lq_sb[:, kk, :],
                            start=(kk == 0), stop=(kk == 3),
                        )
                    qstage = proj.tile([128, QS], BF16, tag="qstage")
                    nc.vector.tensor_copy(qstage[:], ps[:])
                    # shift each head's 64 rows down to base partition 0
                    nc.gpsimd.dma_start(qT_sb[:, 2 * m, :], qstage[0:64, :])
                    nc.gpsimd.dma_start(qT_sb[:, 2 * m + 1, :], qstage[64:128, :])

                # --- k^T projection per t-half; gather kicked per half ---
                for th in range(2):
                    for m in range(4):
                        wcol = proj.tile([128, 32, 128], BF16, tag="wcol")
                        nc.sync.dma_start(wcol[:], w_k_v[:, :, m * 128:(m + 1) * 128])
                        ps = pps.tile([128, 512], F32, tag="pp")
                        for k in range(32):
                            nc.tensor.matmul(
                                ps[:], wcol[:, k, :], tok_sb[:, k, th * 512:(th + 1) * 512],
                                start=(k == 0), stop=(k == 31),
                            )
                        kstage = proj.tile([128, 512], BF16, tag="kstage")
                        nc.vector.tensor_copy(kstage[:], ps[:])
                        nc.sync.dma_start(gk_in_v[th][:, m, :], kstage[:])
                    gather(gk_in[th], gk_out[th])

                # --- v^T projection + PE transpose per t-half; gathered ---
                for th in range(2):
                    for m in range(4):
                        wcol = proj.tile([128, 32, 128], BF16, tag="wcol")
                        nc.sync.dma_start(wcol[:], w_v_v[:, :, m * 128:(m + 1) * 128])
                        ps = pps.tile([128, 512], F32, tag="pp")
                        for k in range(32):
                            nc.tensor.matmul(
                                ps[:], wcol[:, k, :], tok_sb[:, k, th * 512:(th + 1) * 512],
                                start=(k == 0), stop=(k == 31),
                            )
                        vst = proj.tile([128, 512], BF16, tag="vst")
                        nc.vector.tensor_copy(vst[:], ps[:])
                        pt = pps.tile([128, 512], BF16, tag="pt")
                        for j in range(4):
                            nc.tensor.transpose(
                                pt[:, j * 128:(j + 1) * 128],
                                vst[:, j * 128:(j + 1) * 128],
                                idn[:],
                            )
                        # pt columns j hold v[t-chunk j of this half, i-block m]
                        vstage = proj.tile([128, 4, 128], BF16, tag="vstage")
                        nc.vector.tensor_copy(
                            vstage[:], pt[:].rearrange("p (j i) -> p j i", j=4)
                        )
                        nc.sync.dma_start(
                            gv_in_v[th][:, :, m * 128:(m + 1) * 128], vstage[:]
                        )
                    gather(gv_in[th], gv_out[th])

            # ================= phase 2: attention =================
            # gathered views: k per head row-slice; v in processing order
            gk_head = [
                g.rearrange("(c p) t -> p c t", p=INNER) for g in gk_out
            ]                                                       # [512, 8, 512]
            gv_v = [
                g.rearrange("(x p) i -> p x i", p=128) for g in gv_out
            ]                                                       # [128, 32, 512]
            groups = [list(range(s, min(s + GRP, NT))) for s in range(0, NT, GRP)]

            with (
                tc.tile_pool(name="attn", bufs=2) as attn,
                tc.tile_pool(name="attn3", bufs=8) as attn3,
                tc.tile_pool(name="aps", bufs=2, space="PSUM") as aps,
                tc.tile_pool(name="aps1", bufs=1, space="PSUM") as aps1,
            ):
                # all heads' V, position-ordered, loaded in quarters (gpsimd:
                # queued behind the matching gather; quarter q covers
                # positions 16q..16q+16 = cores 4q'..4q'+4 of half q//2)
                vh_all = attn.tile([128, NT, INNER], BF16, tag="vh_all", bufs=1)
                for q in range(4):
                    nc.gpsimd.dma_start(
                        vh_all[:, q * 16:(q + 1) * 16, :],
                        gv_v[q // 2][:, (q % 2) * 16:(q % 2) * 16 + 16, :],
                    )
                # w_out prefetch (no gather dep, but gpsimd is free now)
                wo_all = attn.tile([128, 4, L], BF16, tag="wo_all", bufs=1)
                nc.gpsimd.dma_start(wo_all[:], w_out_v)

                def prefetch_head(h):
                    kTh = attn.tile([64, 2, N_CORES, 512], BF16, tag="kTh")
                    for t in range(2):
                        nc.sync.dma_start(
                            kTh[:, t, :, :], gk_head[t][h * D:(h + 1) * D, :, :]
                        )
                    vh = attn.tile([128, NT, D + 1], BF16, tag="vh")
                    nc.vector.memset(vh[:, :, D], 1.0)
                    for q in range(4):
                        nc.vector.tensor_copy(
                            vh[:, q * 16:(q + 1) * 16, 0:D],
                            vh_all[:, q * 16:(q + 1) * 16, h * D:(h + 1) * D],
                        )
                    return kTh, vh

                def norm_tail(h, u_sb, recip):
                    """Broadcast 1/denom across partitions on gpsimd, rescale
                    on DVE, store. No tensor-engine involvement: the next
                    head's scores never wait on this chain."""
                    rb = attn.tile([D, QS], F32, tag="rb")
                    nc.gpsimd.partition_broadcast(rb[:], recip[:])
                    a_tmp = attn.tile([D, QS], BF16, tag="a_tmp")
                    nc.vector.tensor_mul(a_tmp[:], u_sb[0:D, :], rb[:])
                    nc.gpsimd.dma_start(
                        aT_sb[(h % 2) * 64:(h % 2) * 64 + 64, h // 2, :], a_tmp[:]
                    )

                nxt = prefetch_head(0)
                for h in range(H):
                    kTh, vh = nxt
                    qTh = qT_sb[:, h, :]
                    ps_o = aps1.tile([D + 1, QS], F32, tag="ps_o")
                    lag = 7 if h == 0 else 2
                    pending = []
                    for gi, g in enumerate(groups):
                        ps_s = aps.tile([128, GRP * QS], F32, tag="ps_s")
                        for jj, o in enumerate(g):
                            t, c, jt = ORDER[o]
                            nc.tensor.matmul(
                                ps_s[:, jj * QS:(jj + 1) * QS],
                                kTh[:, t, c, jt * 128:(jt + 1) * 128],
                                qTh,
                                start=True, stop=True,
                            )
                        pT = attn3.tile([128, GRP * QS], BF16, tag="pT")
                        n = len(g) * QS
                        nc.scalar.activation(pT[:, 0:n], ps_s[:, 0:n], EXP, scale=SCALE)
                        pending.append((g, pT))
                        if gi == 2 and h + 1 < H:
                            nxt = prefetch_head(h + 1)
                        if len(pending) > lag:
                            pg, ppT = pending.pop(0)
                            for jj, o in enumerate(pg):
                                nc.tensor.matmul(
                                    ps_o[:], vh[:, o, :], ppT[:, jj * QS:(jj + 1) * QS],
                                    start=(o == 0), stop=(o == NT - 1),
                                    skip_group_check=True,
                                )
                    for pg, ppT in pending:
                        for jj, o in enumerate(pg):
                            nc.tensor.matmul(
                                ps_o[:], vh[:, o, :], ppT[:, jj * QS:(jj + 1) * QS],
                                start=(o == 0), stop=(o == NT - 1),
                                skip_group_check=True,
                            )

                    # u^T and 1/denom now; the broadcast+rescale is deferred
                    # into the next head's score phase
                    u_sb = attn.tile([D + 1, QS], F32, tag="u")
                    nc.vector.tensor_copy(u_sb[:], ps_o[:])
                    dn0 = attn.tile([1, QS], F32, tag="dn0")
                    nc.sync.dma_start(dn0[:], u_sb[D:D + 1, :])  # to partition 0
                    recip = attn.tile([1, QS], F32, tag="recip")
                    nc.vector.reciprocal(recip[:], dn0[:])
                    norm_tail(h, u_sb, recip)

                # ============ phase 3: output projection ============
                for m in range(L // 128):
                    ps = aps.tile([128, QS], F32, tag="ps_s")
                    for kk in range(4):
                        nc.tensor.matmul(
                            ps[:], wo_all[:, kk, m * 128:(m + 1) * 128],
                            aT_sb[:, kk, :],
                            start=(kk == 0), stop=(kk == 3),
                        )
                    of = attn.tile([128, QS], F32, tag="of", bufs=3)
                    nc.vector.tensor_copy(of[:], ps[:])
                    nc.sync.dma_start(outT[m * 128:(m + 1) * 128, :], of[:])

    nc.compile()
    return nc


_COMPILED = None


def _get_compiled():
    global _COMPILED
    if _COMPILED is None:
        _COMPILED = build_program()
    return _COMPILED


def _bf(x):
    return np.ascontiguousarray(np.asarray(x, dtype=np.float32)).astype(
        ml_dtypes.bfloat16
    )


def make_in_maps(token_input, learned_queries, w_q, w_k, w_v, w_out):
    token_input = np.asarray(token_input, dtype=np.float32)
    learned_queries = np.asarray(learned_queries, dtype=np.float32)
    w_q_b, w_k_b, w_v_b, w_out_b = _bf(w_q), _bf(w_k), _bf(w_v), _bf(w_out)
    in_maps = []
    for c in range(N_CORES):
        in_maps.append({
            "tok_T": _bf(token_input[c * TS:(c + 1) * TS, :].T),
            "lq_T": _bf(learned_queries[c * QS:(c + 1) * QS, :].T),
            "w_q": w_q_b, "w_k": w_k_b, "w_v": w_v_b, "w_out": w_out_b,
        })
    return in_maps


def assemble(results):
    out = np.empty((V, L), dtype=np.float32)
    for c in range(N_CORES):
        out[c * QS:(c + 1) * QS, :] = results[c]["outT"].T
    return out


def kernel(token_input, learned_queries, w_q, w_k, w_v, w_out):
    nc = _get_compiled()
    in_maps = make_in_maps(token_input, learned_queries, w_q, w_k, w_v, w_out)
    res = run_bass_kernel_spmd(nc, in_maps, list(range(N_CORES)))
    return assemble(res.results)


# revision 12
# speedup vs baseline: 17.9381x; 3.3237x over previous
"""Cross-attention reducer kernel for Trainium2, 8 NeuronCores (SPMD).

Problem (full shapes):
    token_input    [T=8192, L=4096]
    learned_queries[V=4096, I=512]
    w_q [I, I], w_k [L, I], w_v [L, I], w_out [I, L]

    q = learned_queries @ w_q;  k = token_input @ w_k;  v = token_input @ w_v
    per head h (H=8, D=64): attn = softmax(q_h k_h^T / sqrt(D)); out_h = attn @ v_h
    out = concat_h(out_h) @ w_out      -> [V, L]

Sharding: queries (V) are sharded 8 ways; the K/V projections are
sequence-parallel (each core projects its T/8 token shard) followed by an
AllGather of k^T and v, after which every core runs attention for all 8 heads
over its own 512 queries and the full gathered T, then applies the output
projection for its V-shard. Everything is computed transposed
(final^T = w_out^T-contraction) so every matmul contracts on the partition
dimension with no large transposes anywhere:

    q^T  [I, Vs]  = w_q (lhsT)  x lq^T (rhs)
    k^T  [I, t]   = w_k (lhsT)  x tok^T (rhs)        (gathered)
    v^T  [I, t]   = w_v (lhsT)  x tok^T (rhs), then 128x128 PE-transposes
                    to v [t, I] before the gather
    s^T  [t, Vs]  = k_h^T (lhsT) x q_h^T (rhs)       (t-tiles of 128)
    p^T           = exp(s^T / 8)                      (no max-subtraction:
                    scores are O(3), exp can't overflow; identical math)
    u^T  [D+1,Vs] = [v_h | 1] (lhsT) x p^T (rhs)     (row D = softmax denom)
    a^T  [D, Vs]  = u^T * (1/denom broadcast via PE outer product)
    out^T[L, Vs]  = w_out (lhsT) x a^T (rhs)

All inputs are cast to bf16 on the host (input rounding contributes ~3e-3
max-rel error, tolerance 2e-2); all matmuls run bf16 at 1 cycle/col.

Overlap structure (the point of this version):
  - q projection runs first, under the token-shard load.
  - token shard loaded ONCE into SBUF (bf16), reused by K and V projections.
  - each gather is split into two t-half collectives kicked at the
    projection midpoints, so collective transfer is pipelined with the
    remaining projection work; gathered v is loaded in quarters and the
    attention processes half-a tiles before half-b tiles, so the first
    attn@v only needs the first quarter of the first v-gather.
  - the per-head softmax normalization (denominator broadcast) is deferred
    into the NEXT head's score phase so its serial chain (shift ->
    reciprocal -> PE broadcast) never blocks the in-order tensor queue.
  - collectives + gather-dependent loads (vh_all, w_out prefetch, aT
    stores) issue on gpsimd; all other loads on sync, so a blocking
    collective can never stall the compute-feed queues.
"""

import os

import numpy as np
import ml_dtypes

import concourse.bacc as bacc
import concourse.tile as tile
import concourse.mybir as mybir
from concourse.bass_utils import run_bass_kernel_spmd

F32 = mybir.dt.float32
BF16 = mybir.dt.bfloat16
EXP = mybir.ActivationFunctionType.Exp
EQ = mybir.AluOpType.is_equal

N_CORES = 8
T, L, V, INNER = 8192, 4096, 4096, 512
H, D = 8, 64
TS = T // N_CORES      # 1024  t-shard per core
QS = V // N_CORES      # 512   query shard per core
SCALE = D ** -0.5      # 0.125

NT = T // 128          # 64 gathered t-tiles per head
GRP = 3                # t-tiles per exp batch (3 psum banks)

# processing position o (0..63) -> (th half, core, in-half tile 0..3)
# matches the [half, core, tile] layout of the gathered buffers
ORDER = [(o // 32, (o % 32) // 4, o % 4) for o in range(64)]


def build_program():
    nc = bacc.Bacc(
        "TRN2", target_bir_lowering=False, debug=False, num_devices=N_CORES
    )

    tok_T = nc.dram_tensor("tok_T", [L, TS], BF16, kind="ExternalInput").ap()
    lq_T = nc.dram_tensor("lq_T", [INNER, QS], BF16, kind="ExternalInput").ap()
    w_q = nc.dram_tensor("w_q", [INNER, INNER], BF16, kind="ExternalInput").ap()
    w_k = nc.dram_tensor("w_k", [L, INNER], BF16, kind="ExternalInput").ap()
    w_v = nc.dram_tensor("w_v", [L, INNER], BF16, kind="ExternalInput").ap()
    w_out = nc.dram_tensor("w_out", [INNER, L], BF16, kind="ExternalInput").ap()
    outT = nc.dram_tensor("outT", [L, QS], F32, kind="ExternalOutput").ap()

    # rearranged DRAM views (partition-major for SBUF loads)
    tok_v = tok_T.rearrange("(k p) t -> p k t", p=128)          # [128, 32, 1024]
    lq_v = lq_T.rearrange("(k p) q -> p k q", p=128)            # [128, 4, 512]
    w_q_v = w_q.rearrange("(k p) i -> p k i", p=128)            # [128, 4, 512]
    w_k_v = w_k.rearrange("(k p) i -> p k i", p=128)            # [128, 32, 512]
    w_v_v = w_v.rearrange("(k p) i -> p k i", p=128)            # [128, 32, 512]
    w_out_v = w_out.rearrange("(k p) l -> p k l", p=128)        # [128, 4, 4096]

    no_cc = bool(os.environ.get("BASSK_NO_CC"))

    with tile.TileContext(nc) as tc:
        with (
            tc.tile_pool(name="persist", bufs=1) as persist,
            tc.tile_pool(name="dram", bufs=1, space="DRAM") as dram,
        ):
            # ---- persistent SBUF across phases ----
            qT_sb = persist.tile([64, H, QS], BF16, tag="qT")        # q^T per head
            aT_sb = persist.tile([128, 4, QS], BF16, tag="aT")       # attn out^T
            idn = persist.tile([128, 128], BF16, tag="idn")          # identity
            ones_64 = persist.tile([1, D], F32, tag="ones64")

            # collective bounce buffers, one pair per t-half
            gk_in = [dram.tile([INNER, 512], BF16, tag=f"gk_in{t}", name=f"gk_in{t}") for t in range(2)]
            gk_out = [
                dram.tile([N_CORES * INNER, 512], BF16, tag=f"gk_out{t}",
                          name=f"gk_out{t}", addr_space="Shared")
                for t in range(2)
            ]
            gv_in = [dram.tile([512, INNER], BF16, tag=f"gv_in{t}", name=f"gv_in{t}") for t in range(2)]
            gv_out = [
                dram.tile([N_CORES * 512, INNER], BF16, tag=f"gv_out{t}",
                          name=f"gv_out{t}", addr_space="Shared")
                for t in range(2)
            ]
            gk_in_v = [g.rearrange("(m p) t -> p m t", p=128) for g in gk_in]
            gv_in_v = [g.rearrange("(j p) i -> p j i", p=128) for g in gv_in]

            def gather(src, dst):
                if no_cc:
                    nc.sync.dma_start(dst[0:src.shape[0], :], src[:])
                else:
                    nc.gpsimd.collective_compute(
                        "AllGather", mybir.AluOpType.bypass,
                        replica_groups=[list(range(N_CORES))],
                        ins=[src.opt()], outs=[dst.opt()],
                    )

            # identity matrix for PE transposes: idn[p, f] = (f == p)
            with tc.tile_pool(name="idpool", bufs=1) as idp:
                irow = idp.tile([128, 128], F32, tag="irow")
                icol = idp.tile([128, 1], F32, tag="icol")
                nc.gpsimd.iota(irow[:], pattern=[[1, 128]], base=0, channel_multiplier=0, allow_small_or_imprecise_dtypes=True)
                nc.gpsimd.iota(icol[:], pattern=[[0, 1]], base=0, channel_multiplier=1, allow_small_or_imprecise_dtypes=True)
                nc.vector.tensor_scalar(idn[:], irow[:], icol[:], None, EQ)
            nc.vector.memset(ones_64[:], 1.0)

            # ================= phase 1: projections =================
            with (
                tc.tile_pool(name="ptok", bufs=1) as ptok,
                tc.tile_pool(name="proj", bufs=2) as proj,
                tc.tile_pool(name="pps", bufs=2, space="PSUM") as pps,
            ):
                # --- q^T projection first (PE warms up under the tok load) ---
                wq_sb = proj.tile([128, 4, INNER], BF16, tag="wq", bufs=1)
                lq_sb = proj.tile([128, 4, QS], BF16, tag="lq", bufs=1)
                nc.sync.dma_start(wq_sb[:], w_q_v)
                nc.sync.dma_start(lq_sb[:], lq_v)

                # token shard resident in SBUF, loaded once (2 chunked DMAs)
                tok_sb = ptok.tile([128, 32, TS], BF16, tag="tok")
                nc.sync.dma_start(tok_sb[:, :, 0:512], tok_v[:, :, 0:512])
                nc.sync.dma_start(tok_sb[:, :, 512:1024], tok_v[:, :, 512:1024])
                # whole-weight loads: few big DMAs beat many small ones
                wk_sb = proj.tile([128, 32, 512], BF16, tag="wkv")
                nc.sync.dma_start(wk_sb[:], w_k_v)
                wv_sb = proj.tile([128, 32, 512], BF16, tag="wkv")
                nc.sync.dma_start(wv_sb[:], w_v_v)

                for m in range(4):
                    ps = pps.tile([128, QS], F32, tag="pp")
                    for kk in range(4):
                        nc.tensor.matmul(
                            ps[:],
                            wq_sb[:, kk, m * 128:(m + 1) * 128],
                            lq_sb[:, kk, :],
                            start=(kk == 0), stop=(kk == 3),
                        )
                    qstage = proj.tile([128, QS], BF16, tag="qstage")
                    nc.vector.tensor_copy(qstage[:], ps[:])
                    # shift each head's 64 rows down to base partition 0
                    nc.gpsimd.dma_start(qT_sb[:, 2 * m, :], qstage[0:64, :])
                    nc.gpsimd.dma_start(qT_sb[:, 2 * m + 1, :], qstage[64:128, :])

                # --- k^T projection per t-half; gather kicked per half ---
                for th in range(2):
                    kstage = proj.tile([128, 4, 512], BF16, tag="kstage")
                    for m in range(4):
                        ps = pps.tile([128, 512], F32, tag="pp")
                        for k in range(32):
                            nc.tensor.matmul(
                                ps[:], wk_sb[:, k, m * 128:(m + 1) * 128],
                                tok_sb[:, k, th * 512:(th + 1) * 512],
                                start=(k == 0), stop=(k == 31),
                            )
                        nc.vector.tensor_copy(kstage[:, m, :], ps[:])
                    nc.sync.dma_start(gk_in_v[th][:], kstage[:])
                    gather(gk_in[th], gk_out[th])

                # --- v^T projection + PE transpose per t-half; gathered ---
                for th in range(2):
                    vstage = proj.tile([128, 4, 512], BF16, tag="vstage")
                    for m in range(4):
                        ps = pps.tile([128, 512], F32, tag="pp")
                        for k in range(32):
                            nc.tensor.matmul(
                                ps[:], wv_sb[:, k, m * 128:(m + 1) * 128],
                                tok_sb[:, k, th * 512:(th + 1) * 512],
                                start=(k == 0), stop=(k == 31),
                            )
                        vst = proj.tile([128, 512], BF16, tag="vst")
                        nc.vector.tensor_copy(vst[:], ps[:])
                        pt = pps.tile([128, 512], BF16, tag="pt")
                        for j in range(4):
                            nc.tensor.transpose(
                                pt[:, j * 128:(j + 1) * 128],
                                vst[:, j * 128:(j + 1) * 128],
                                idn[:],
                            )
                        # pt columns j hold v[t-chunk j of this half, i-block m]
                        nc.vector.tensor_copy(
                            vstage[:, :, m * 128:(m + 1) * 128],
                            pt[:].rearrange("p (j i) -> p j i", j=4),
                        )
                    nc.sync.dma_start(gv_in_v[th][:], vstage[:])
                    gather(gv_in[th], gv_out[th])

            # ================= phase 2: attention =================
            # gathered views: k per head row-slice; v in processing order
            gk_head = [
                g.rearrange("(c p) t -> p c t", p=INNER) for g in gk_out
            ]                                                       # [512, 8, 512]
            gv_v = [
                g.rearrange("(x p) i -> p x i", p=128) for g in gv_out
            ]                                                       # [128, 32, 512]
            groups = [list(range(s, min(s + GRP, NT))) for s in range(0, NT, GRP)]

            with (
                tc.tile_pool(name="attn", bufs=2) as attn,
                tc.tile_pool(name="attn3", bufs=8) as attn3,
                tc.tile_pool(name="aps", bufs=2, space="PSUM") as aps,
                tc.tile_pool(name="aps1", bufs=1, space="PSUM") as aps1,
            ):
                # all heads' V, position-ordered, loaded in quarters (gpsimd:
                # queued behind the matching gather; quarter q covers
                # positions 16q..16q+16 = cores 4q'..4q'+4 of half q//2)
                vh_all = attn.tile([128, NT, INNER], BF16, tag="vh_all", bufs=1)
                for q in range(4):
                    nc.gpsimd.dma_start(
                        vh_all[:, q * 16:(q + 1) * 16, :],
                        gv_v[q // 2][:, (q % 2) * 16:(q % 2) * 16 + 16, :],
                    )
                # w_out prefetch (no gather dep, but gpsimd is free now)
                wo_all = attn.tile([128, 4, L], BF16, tag="wo_all", bufs=1)
                nc.gpsimd.dma_start(wo_all[:], w_out_v)

                def prefetch_head(h):
                    kTh = attn.tile([64, 2, N_CORES, 512], BF16, tag="kTh")
                    for t in range(2):
                        nc.sync.dma_start(
                            kTh[:, t, :, :], gk_head[t][h * D:(h + 1) * D, :, :]
                        )
                    vh = attn.tile([128, NT, D + 1], BF16, tag="vh")
                    nc.vector.memset(vh[:, :, D], 1.0)
                    for q in range(4):
                        nc.vector.tensor_copy(
                            vh[:, q * 16:(q + 1) * 16, 0:D],
                            vh_all[:, q * 16:(q + 1) * 16, h * D:(h + 1) * D],
                        )
                    return kTh, vh

                def norm_tail(h, u_sb, recip):
                    """Broadcast 1/denom across partitions on gpsimd, rescale
                    on DVE, store. No tensor-engine involvement: the next
                    head's scores never wait on this chain."""
                    rb = attn.tile([D, QS], F32, tag="rb")
                    nc.gpsimd.partition_broadcast(rb[:], recip[:])
                    a_tmp = attn.tile([D, QS], BF16, tag="a_tmp")
                    nc.vector.tensor_mul(a_tmp[:], u_sb[0:D, :], rb[:])
                    nc.gpsimd.dma_start(
                        aT_sb[(h % 2) * 64:(h % 2) * 64 + 64, h // 2, :], a_tmp[:]
                    )

                nxt = prefetch_head(0)
                for h in range(H):
                    kTh, vh = nxt
                    qTh = qT_sb[:, h, :]
                    ps_o = aps1.tile([D + 1, QS], F32, tag="ps_o")
                    lag = 7 if h == 0 else 2
                    pending = []
                    for gi, g in enumerate(groups):
                        ps_s = aps.tile([128, GRP * QS], F32, tag="ps_s")
                        for jj, o in enumerate(g):
                            t, c, jt = ORDER[o]
                            nc.tensor.matmul(
                                ps_s[:, jj * QS:(jj + 1) * QS],
                                kTh[:, t, c, jt * 128:(jt + 1) * 128],
                                qTh,
                                start=True, stop=True,
                            )
                        pT = attn3.tile([128, GRP * QS], BF16, tag="pT")
                        n = len(g) * QS
                        nc.scalar.activation(pT[:, 0:n], ps_s[:, 0:n], EXP, scale=SCALE)
                        pending.append((g, pT))
                        if gi == 2 and h + 1 < H:
                            nxt = prefetch_head(h + 1)
                        if len(pending) > lag:
                            pg, ppT = pending.pop(0)
                            for jj, o in enumerate(pg):
                                nc.tensor.matmul(
                                    ps_o[:], vh[:, o, :], ppT[:, jj * QS:(jj + 1) * QS],
                                    start=(o == 0), stop=(o == NT - 1),
                                    skip_group_check=True,
                                )
                    for pg, ppT in pending:
                        for jj, o in enumerate(pg):
                            nc.tensor.matmul(
                                ps_o[:], vh[:, o, :], ppT[:, jj * QS:(jj + 1) * QS],
                                start=(o == 0), stop=(o == NT - 1),
                                skip_group_check=True,
                            )

                    # u^T and 1/denom now; the broadcast+rescale is deferred
                    # into the next head's score phase
                    u_sb = attn.tile([D + 1, QS], F32, tag="u")
                    nc.vector.tensor_copy(u_sb[:], ps_o[:])
                    dn0 = attn.tile([1, QS], F32, tag="dn0")
                    nc.sync.dma_start(dn0[:], u_sb[D:D + 1, :])  # to partition 0
                    recip = attn.tile([1, QS], F32, tag="recip")
                    nc.vector.reciprocal(recip[:], dn0[:])
                    norm_tail(h, u_sb, recip)

                # ============ phase 3: output projection ============
                outT_v = outT.rearrange("(m p) q -> p m q", p=128)   # [128, 32, 512]
                for mb in range(16):
                    of = attn.tile([128, 2, QS], F32, tag="of", bufs=2)
                    for mi in range(2):
                        m = mb * 2 + mi
                        ps = aps.tile([128, QS], F32, tag="ps_s")
                        for kk in range(4):
                            nc.tensor.matmul(
                                ps[:], wo_all[:, kk, m * 128:(m + 1) * 128],
                                aT_sb[:, kk, :],
                                start=(kk == 0), stop=(kk == 3),
                            )
                        nc.vector.tensor_copy(of[:, mi, :], ps[:])
                    nc.sync.dma_start(outT_v[:, mb * 2:(mb + 1) * 2, :], of[:])

    nc.compile()
    return nc


_COMPILED = None


def _get_compiled():
    global _COMPILED
    if _COMPILED is None:
        _COMPILED = build_program()
    return _COMPILED


def _bf(x):
    return np.ascontiguousarray(np.asarray(x, dtype=np.float32)).astype(
        ml_dtypes.bfloat16
    )


def make_in_maps(token_input, learned_queries, w_q, w_k, w_v, w_out):
    token_input = np.asarray(token_input, dtype=np.float32)
    learned_queries = np.asarray(learned_queries, dtype=np.float32)
    w_q_b, w_k_b, w_v_b, w_out_b = _bf(w_q), _bf(w_k), _bf(w_v), _bf(w_out)
    in_maps = []
    for c in range(N_CORES):
        in_maps.append({
            "tok_T": _bf(token_input[c * TS:(c + 1) * TS, :].T),
            "lq_T": _bf(learned_queries[c * QS:(c + 1) * QS, :].T),
            "w_q": w_q_b, "w_k": w_k_b, "w_v": w_v_b, "w_out": w_out_b,
        })
    return in_maps


def assemble(results):
    out = np.empty((V, L), dtype=np.float32)
    for c in range(N_CORES):
        out[c * QS:(c + 1) * QS, :] = results[c]["outT"].T
    return out


def kernel(token_input, learned_queries, w_q, w_k, w_v, w_out):
    nc = _get_compiled()
    in_maps = make_in_maps(token_input, learned_queries, w_q, w_k, w_v, w_out)
    res = run_bass_kernel_spmd(nc, in_maps, list(range(N_CORES)))
    return assemble(res.results)
